# revision 11
# baseline (speedup 1.0000x reference)
"""Trainium2 Bass kernel for nn_DynamicsShaper: time-varying RBJ lowpass biquad
driven by per-segment-averaged logits.

Sharding: batch row r -> NeuronCore r (8 rows, 8 cores, fully independent).

Per-core layout: the row of T=160000 samples is viewed as [128 partitions x
W=1250].  First-order recurrences (segmented cumsum for run means, reverse
hold-scan for broadcast) use the DVE TensorTensorScan instruction per
partition, chained across partitions via a PE transpose + a [.,128] scan.
The order-2 IIR uses a blocked scan: C=25 chunks of L=50 per partition run
three coupled recursions (zero-state response + two homogeneous solutions)
in lockstep, then chunk-to-chunk affine state maps are combined by a
3-basis walk within each partition and a log2(128)-round Hillis-Steele
(PE shift matrices) across partitions, followed by a linear correction pass.
"""

import sys

sys.path.insert(0, "/opt/trn_rl_repo")

import numpy as np

import concourse.bass as bass
import concourse.bacc as bacc
import concourse.mybir as mybir
import concourse.tile as tile
from concourse import masks

P = 128          # SBUF partitions
W = 1250         # samples per partition (T = P*W)
C = 25           # chunks per partition
L = W // C       # chunk length (50)
T = P * W
B = 8
SR = 16000.0
GAIN_MIN, GAIN_MAX = 0.1, 2.0
LOG_MIN_W = float(np.log(2.0 * np.pi * 20.0 / SR))
LOG_MAX_W = float(np.log(np.pi))
LOG_MIN_Q, LOG_MAX_Q = float(np.log(0.0707)), float(np.log(2.0))

fp = mybir.dt.float32
i32 = mybir.dt.int32
OP = mybir.AluOpType
AF = mybir.ActivationFunctionType


def build_program():
    nc = bacc.Bacc("TRN2", target_bir_lowering=False, debug=False, num_devices=B)
    d_noise = nc.dram_tensor("noise", [P, W], fp, kind="ExternalInput").ap()
    d_seg = nc.dram_tensor("seg", [P, W], i32, kind="ExternalInput").ap()
    d_logits = nc.dram_tensor("logits", [P, 3 * W], fp, kind="ExternalInput").ap()
    d_y = nc.dram_tensor("y", [P, W], fp, kind="ExternalOutput").ap()
    with tile.TileContext(nc) as tc:
        _body(nc, tc, d_noise, d_seg, d_logits, d_y)
    nc.compile()
    return nc


def _body(nc, tc, d_noise, d_seg, d_logits, d_y):
    from contextlib import ExitStack
    ctx = ExitStack()
    pool = ctx.enter_context(tc.tile_pool(name="main", bufs=1))
    psum = ctx.enter_context(tc.tile_pool(name="ps", bufs=1, space="PSUM"))

    V = nc.vector
    G = nc.gpsimd
    A = nc.scalar

    # ---------- loads ----------
    seg = pool.tile([P, W], i32)
    logits = pool.tile([P, 3 * W], fp)
    noise = pool.tile([P, W], fp)
    nc.sync.dma_start(seg[:], d_seg)
    nc.sync.dma_start(logits[:], d_logits)
    nc.sync.dma_start(noise[:], d_noise)

    # ---------- constants: identity + shift matrices ----------
    ident = pool.tile([P, P], fp)
    masks.make_identity(nc, ident[:])
    ident8 = pool.tile([8, 8], fp)
    masks.make_identity(nc, ident8[:])

    def shift_mat(base):
        m = pool.tile([P, P], fp, name=f"shift_{base}")
        G.memset(m[:], 0.0)
        G.affine_select(out=m[:], in_=m[:], compare_op=OP.not_equal, fill=1.0,
                        base=base, pattern=[[-1, P]], channel_multiplier=1)
        return m

    sh_up = {s: shift_mat(s) for s in (1, 2, 4, 8, 16, 32, 64)}  # out[p] = in[p-s]
    sh_dn = shift_mat(-1)                                        # out[p] = in[p+1]

    # identity-affine pads for HS rounds: rows < s get identity map
    # map layout per 6 cols: (d1, p1, q1, d2, p2, q2); identity: p1=1, q2=1
    idpad = {}
    for s in (1, 2, 4, 8, 16, 32, 64):
        t = pool.tile([P, 6], fp, name=f"idpad_{s}")
        V.memset(t[:], 0.0)
        V.memset(t[0:s, 1:2], 1.0)
        V.memset(t[0:s, 5:6], 1.0)
        idpad[s] = t

    ones = pool.tile([P, W], fp)
    V.memset(ones[:], 1.0)

    # ---------- boundary columns via PE shifts ----------
    # staging cols: (x is not ready yet; noise cols are NOT what we need --
    # x = noise*gain, but gain is per-run so x boundary needs gain too.
    # We shift noise cols now and multiply by shifted gain later?  gain at
    # (p-1, W-1) equals... simpler: shift x cols after x computed. Here we
    # only shift the seg boundary columns.)
    segf = pool.tile([P, 2], fp)  # fp casts: col0 = seg[:,0], col1 = seg[:,W-1]
    V.tensor_copy(segf[:, 0:1], seg[:, 0:1])
    V.tensor_copy(segf[:, 1:2], seg[:, W - 1:W])
    ps_b = psum.tile([P, 2], fp, tag="ps_small")
    nc.tensor.matmul(ps_b[:, 0:1], sh_up[1][:], segf[:, 1:2])   # seg[p-1, W-1]
    nc.tensor.matmul(ps_b[:, 1:2], sh_dn[:], segf[:, 0:1])      # seg[p+1, 0]
    bnd = pool.tile([P, 2], fp)
    V.tensor_copy(bnd[:], ps_b[:])
    V.memset(bnd[0:1, 0:1], -1.0)     # row 0 has no predecessor
    # row 127 has no successor: fill -1 where partition == 127 (quadrant rule
    # forbids a direct memset at partition 127)
    G.affine_select(out=bnd[:, 1:2], in_=bnd[:, 1:2], compare_op=OP.not_equal,
                    fill=-1.0, base=-(P - 1), pattern=[[0, 1]],
                    channel_multiplier=1)

    # ---------- gates ----------
    # cmp[P, W+1]: cmp[:, j] (1<=j<=W-1) = (seg[j] == seg[j-1]); col 0 = gate
    # at partition start; col W = "continues into next partition".
    # g = cmp[:, 0:W] (gate for forward scans), e = cmp[:, 1:W+1] (1 - is_end).
    cmp = pool.tile([P, W + 1], fp)
    V.tensor_tensor(cmp[:, 1:W], seg[:, 1:], seg[:, :W - 1], OP.is_equal)
    V.tensor_tensor(cmp[:, 0:1], segf[:, 0:1], bnd[:, 0:1], OP.is_equal)
    V.tensor_tensor(cmp[:, W:W + 1], segf[:, 1:2], bnd[:, 1:2], OP.is_equal)
    g = cmp[:, 0:W]
    e = cmp[:, 1:W + 1]

    # ---------- forward segmented scans (zero init) ----------
    d0 = [pool.tile([P, W], fp, name=f"d0_{c}") for c in range(3)]
    l0 = pool.tile([P, W], fp)
    for c in range(3):
        V.tensor_tensor_scan(d0[c][:], g, logits[:, c::3], 0.0, OP.mult, OP.add)
    V.tensor_tensor_scan(l0[:], g, ones[:], 0.0, OP.mult, OP.add)

    gP = pool.tile([P, 2], fp)   # col0: prod(g) per partition, col1: prod(e)
    V.tensor_reduce(gP[:, 0:1], g, mybir.AxisListType.X, OP.min)
    V.tensor_reduce(gP[:, 1:2], e, mybir.AxisListType.X, OP.min)

    # ---------- cross-partition chain for forward scans ----------
    # summaries [P, 8]: (gP, gP, gP, gP, d0_0[W-1], d0_1[W-1], d0_2[W-1], l0[W-1])
    s8 = pool.tile([P, 8], fp)
    V.tensor_copy(s8[:, 0:4], gP[:, 0:1].to_broadcast([P, 4]))
    for c in range(3):
        V.tensor_copy(s8[:, 4 + c:5 + c], d0[c][:, W - 1:W])
    V.tensor_copy(s8[:, 7:8], l0[:, W - 1:W])
    ps_tg = psum.tile([4, P], fp, tag="ps_a")
    ps_td = psum.tile([4, P], fp, tag="ps_bb")
    nc.tensor.transpose(ps_tg[:], s8[:, 0:4], ident[:])
    nc.tensor.transpose(ps_td[:], s8[:, 4:8], ident[:])
    t8g = pool.tile([4, P], fp)
    t8d = pool.tile([4, P], fp)
    V.tensor_copy(t8g[:], ps_tg[:])
    V.tensor_copy(t8d[:], ps_td[:])
    ch = pool.tile([4, P], fp)
    V.tensor_tensor_scan(ch[:], t8g[:], t8d[:], 0.0, OP.mult, OP.add)
    chs = pool.tile([4, P], fp)   # exclusive: shift right by one, col0 = 0
    V.memset(chs[:, 0:1], 0.0)
    V.tensor_copy(chs[:, 1:P], ch[:, 0:P - 1])
    ps_c = psum.tile([P, 4], fp, tag="ps_cc")
    nc.tensor.matmul(ps_c[:], chs[:], ident8[0:4, 0:4])
    dIn = pool.tile([P, 4], fp)
    V.tensor_copy(dIn[:], ps_c[:])

    # ---------- forward rescans with true initial state ----------
    d = d0  # overwrite in place (summaries already consumed)
    for c in range(3):
        V.tensor_tensor_scan(d[c][:], g, logits[:, c::3], dIn[:, c:c + 1],
                             OP.mult, OP.add)
    l = l0
    V.tensor_tensor_scan(l[:], g, ones[:], dIn[:, 3:4], OP.mult, OP.add)

    # ---------- run means at run-ends; reverse hold-scan broadcast ----------
    rl = pool.tile([P, W], fp)
    V.reciprocal(rl[:], l[:])
    ie = pool.tile([P, W], fp)
    V.tensor_scalar(ie[:], e, -1.0, 1.0, OP.mult, OP.add)   # is_end
    h = ie
    V.tensor_tensor(h[:], ie[:], rl[:], OP.mult)            # in place: ie*rl
    dat = [pool.tile([P, W], fp, name=f"dat_{c}") for c in range(3)]
    for c in range(3):
        V.tensor_tensor(dat[c][:], d[c][:], h[:], OP.mult)
    m0 = [pool.tile([P, W], fp, name=f"m0_{c}") for c in range(3)]
    for c in range(3):
        V.tensor_tensor_scan(m0[c][:, ::-1], e[:, ::-1], dat[c][:, ::-1],
                             0.0, OP.mult, OP.add)
    # reverse chain across partitions (descending p)
    s8r = pool.tile([P, 8], fp)
    V.tensor_copy(s8r[:, 0:4], gP[:, 1:2].to_broadcast([P, 4]))
    for c in range(3):
        V.tensor_copy(s8r[:, 4 + c:5 + c], m0[c][:, 0:1])
    V.memset(s8r[:, 7:8], 0.0)
    ps_t2g = psum.tile([4, P], fp, tag="ps_a")
    ps_t2d = psum.tile([4, P], fp, tag="ps_bb")
    nc.tensor.transpose(ps_t2g[:], s8r[:, 0:4], ident[:])
    nc.tensor.transpose(ps_t2d[:], s8r[:, 4:8], ident[:])
    t8rg = pool.tile([4, P], fp)
    t8rd = pool.tile([4, P], fp)
    V.tensor_copy(t8rg[:], ps_t2g[:])
    V.tensor_copy(t8rd[:], ps_t2d[:])
    chr_ = pool.tile([4, P], fp)
    V.tensor_tensor_scan(chr_[:, ::-1], t8rg[:, ::-1], t8rd[:, ::-1],
                         0.0, OP.mult, OP.add)
    chrs = pool.tile([4, P], fp)  # mIn[p] = chr_[p+1], col W-1... col P-1 = 0
    V.memset(chrs[:, P - 1:P], 0.0)
    V.tensor_copy(chrs[:, 0:P - 1], chr_[:, 1:P])
    ps_c2 = psum.tile([P, 4], fp, tag="ps_cc")
    nc.tensor.matmul(ps_c2[:], chrs[:], ident8[0:4, 0:4])
    mIn = pool.tile([P, 4], fp)
    V.tensor_copy(mIn[:], ps_c2[:])
    # means, broadcast over runs (in place over m0)
    m = m0
    for c in range(3):
        V.tensor_tensor_scan(m[c][:, ::-1], e[:, ::-1], dat[c][:, ::-1],
                             mIn[:, c:c + 1], OP.mult, OP.add)

    # ---------- coefficients ----------
    sg = dat  # reuse: dat tiles dead after the m rescans
    for c in range(3):
        A.activation(sg[c][:], m[c][:], AF.Sigmoid)
    bias_w = pool.tile([P, 1], fp)
    V.memset(bias_w[:], LOG_MIN_W)
    bias_q = pool.tile([P, 1], fp)
    V.memset(bias_q[:], -LOG_MIN_Q)
    bias_hp = pool.tile([P, 1], fp)
    V.memset(bias_hp[:], float(np.pi / 2))
    w = d0[1]  # d tiles dead after dat computed
    A.activation(w[:], sg[1][:], AF.Exp, bias=bias_w[:],
                 scale=(LOG_MAX_W - LOG_MIN_W))
    qinv = d0[2]
    A.activation(qinv[:], sg[2][:], AF.Exp, bias=bias_q[:],
                 scale=-(LOG_MAX_Q - LOG_MIN_Q))
    sinw = d0[0]
    A.activation(sinw[:], w[:], AF.Sin)
    cosw = l0  # dead after rl
    A.activation(cosw[:], w[:], AF.Sin, bias=bias_hp[:], scale=-1.0)
    alpha = rl  # dead after h
    V.scalar_tensor_tensor(alpha[:], sinw[:], 0.5, qinv[:], OP.mult, OP.mult)
    a0 = ones  # dead after l rescan
    V.tensor_scalar(a0[:], alpha[:], 1.0, None, OP.add)
    inva0 = None  # reuse cmp's storage (dead after m rescans)
    inva0 = cmp
    V.reciprocal(inva0[:, 0:W], a0[:])
    b0 = pool.tile([P, W], fp)
    V.tensor_scalar(b0[:], cosw[:], -0.5, 0.5, OP.mult, OP.add)
    V.tensor_tensor(b0[:], b0[:], inva0[:, 0:W], OP.mult)
    na1 = pool.tile([P, W], fp)
    V.scalar_tensor_tensor(na1[:], cosw[:], 2.0, inva0[:, 0:W], OP.mult, OP.mult)
    na2 = pool.tile([P, W], fp)
    V.scalar_tensor_tensor(na2[:], alpha[:], 1.0, inva0[:, 0:W], OP.subtract, OP.mult)
    gain = ie  # h dead after dat
    V.tensor_scalar(gain[:], sg[0][:], GAIN_MAX - GAIN_MIN, GAIN_MIN,
                    OP.mult, OP.add)
    x = m0[0]  # m dead after sigmoids
    V.tensor_tensor(x[:], noise[:], gain[:], OP.mult)

    # ---------- FIR part: f = b0 * (x + 2*x[-1] + x[-2]) ----------
    ps_x = psum.tile([P, 2], fp, tag="ps_small")
    nc.tensor.matmul(ps_x[:], sh_up[1][:], x[:, W - 2:W])
    xb = pool.tile([P, 2], fp)   # (x[p-1, W-2], x[p-1, W-1]); row0 = 0
    V.tensor_copy(xb[:], ps_x[:])
    s_f = m0[1]
    V.scalar_tensor_tensor(s_f[:, 2:], x[:, 1:W - 1], 2.0, x[:, 2:], OP.mult, OP.add)
    f = m0[2]
    V.tensor_tensor(f[:, 2:], s_f[:, 2:], x[:, :W - 2], OP.add)
    # boundary cols 0,1
    V.scalar_tensor_tensor(s_f[:, 0:1], xb[:, 1:2], 2.0, x[:, 0:1], OP.mult, OP.add)
    V.tensor_tensor(f[:, 0:1], s_f[:, 0:1], xb[:, 0:1], OP.add)
    V.scalar_tensor_tensor(s_f[:, 1:2], x[:, 0:1], 2.0, x[:, 1:2], OP.mult, OP.add)
    V.tensor_tensor(f[:, 1:2], s_f[:, 1:2], xb[:, 1:2], OP.add)
    V.tensor_tensor(f[:], f[:], b0[:], OP.mult)  # in place scale

    # ---------- packed recursion coefficients: per (c, n): (-a2, -a1) ----------
    cpk = pool.tile([P, 2 * W], fp)
    cpk4 = cpk.rearrange("p (c n k) -> p c n k", c=C, n=L, k=2)
    na2v = na2.rearrange("p (c n) -> p c n", c=C)
    na1v = na1.rearrange("p (c n) -> p c n", c=C)
    V.tensor_copy(cpk4[:, :, :, 0:1], na2v.unsqueeze(3))
    V.tensor_copy(cpk4[:, :, :, 1:2], na1v.unsqueeze(3))

    # ---------- within-chunk recursions (y_zs, p, q interleaved) ----------
    # ypq[P, C, (L+2)*3]: slot k holds 3 values (y, p, q) for recursion index
    # k-2; slots 0,1 are the initial conditions.
    ypq = pool.tile([P, C * (L + 2) * 3], fp)
    ypq3 = ypq.rearrange("p (c m) -> p c m", c=C)
    V.memset(ypq3[:, :, 0:6], 0.0)
    V.memset(ypq3[:, :, 2:3], 1.0)   # q_{-2} = 1
    V.memset(ypq3[:, :, 4:5], 1.0)   # p_{-1} = 1
    u = pool.tile([P, C * 6], fp)
    u4 = u.rearrange("p (c a b) -> p c a b", c=C, a=2, b=3)
    u3 = u.rearrange("p (c m) -> p c m", c=C)
    f3 = f.rearrange("p (c n) -> p c n", c=C)
    for n in range(L):
        prevs = ypq3[:, :, 3 * n:3 * n + 6].rearrange(
            "p c (a b) -> p c a b", a=2, b=3)
        coef = cpk4[:, :, n:n + 1, :].transpose([0, 1, 3, 2]).to_broadcast(
            [P, C, 2, 3])
        V.tensor_tensor(u4[:], prevs, coef, OP.mult)
        V.tensor_tensor(ypq3[:, :, 3 * n + 6:3 * n + 9], u3[:, :, 0:3],
                        u3[:, :, 3:6], OP.add)
        V.tensor_tensor(ypq3[:, :, 3 * n + 6:3 * n + 7],
                        ypq3[:, :, 3 * n + 6:3 * n + 7],
                        f3[:, :, n:n + 1], OP.add)

    # ---------- 3-basis chunk walk within each partition ----------
    # state slot pair order: (beta, alpha) = (y_{-2}, y_{-1}); walks: 0 = zero
    # state, 1 = alpha basis, 2 = beta basis.
    S = pool.tile([P, 3 * (C + 1) * 2], fp)
    S4 = S.rearrange("p (w s k) -> p w s k", w=3, s=C + 1, k=2)
    V.memset(S[:], 0.0)
    V.memset(S4[:, 1:2, 0:1, 1:2], 1.0)
    V.memset(S4[:, 2:3, 0:1, 0:1], 1.0)
    wk = pool.tile([P, 6], fp)
    wk4 = wk.rearrange("p (w s k) -> p w s k", w=3, s=1, k=2)
    wk4b = pool.tile([P, 6], fp)
    wkb4 = wk4b.rearrange("p (w s k) -> p w s k", w=3, s=1, k=2)
    base = 3 * L
    for c in range(C):
        Wp = ypq3[:, c:c + 1, base + 1:base + 5:3].unsqueeze(1).to_broadcast(
            [P, 3, 1, 2])
        Wq = ypq3[:, c:c + 1, base + 2:base + 6:3].unsqueeze(1).to_broadcast(
            [P, 3, 1, 2])
        dp = ypq3[:, c:c + 1, base:base + 4:3].unsqueeze(1).to_broadcast(
            [P, 3, 1, 2])
        al = S4[:, :, c:c + 1, 1:2].to_broadcast([P, 3, 1, 2])
        be = S4[:, :, c:c + 1, 0:1].to_broadcast([P, 3, 1, 2])
        V.tensor_tensor(wk4[:], Wp, al, OP.mult)
        V.tensor_tensor(wkb4[:], Wq, be, OP.mult)
        V.tensor_tensor(wk4[:], wk4[:], wkb4[:], OP.add)
        V.tensor_tensor(S4[:, :, c + 1:c + 2, :], wk4[:], dp, OP.add)

    # ---------- partition-level affine maps ----------
    # Mcur[P, 6] = (d1, p1, q1, d2, p2, q2):  alpha' = p1 a + q1 b + d1 etc.
    Mcur = pool.tile([P, 6], fp)
    Snap = S4[:, :, C:C + 1, :]  # [P, 3, 1, 2]
    for row, comp in ((0, 1), (1, 0)):  # row 0: alpha (k=1), row 1: beta (k=0)
        sv = Snap[:, :, :, comp:comp + 1].rearrange("p a b c -> p (a b c)")
        dsc = Snap[:, 0:1, :, comp:comp + 1].rearrange(
            "p a b c -> p (a b c)").to_broadcast([P, 3])
        V.tensor_tensor(Mcur[:, 3 * row:3 * row + 3], sv, dsc, OP.subtract)
        V.tensor_copy(Mcur[:, 3 * row:3 * row + 1],
                      Snap[:, 0:1, :, comp:comp + 1].rearrange(
                          "p a b c -> p (a b c)"))

    # ---------- Hillis-Steele inclusive scan of affine maps over partitions ----
    Mnew = pool.tile([P, 6], fp)
    ash = pool.tile([P, 6], fp)
    v6 = pool.tile([P, 6], fp)
    u1t = pool.tile([P, 6], fp)
    u2t = pool.tile([P, 6], fp)
    ps_m = psum.tile([P, 6], fp)
    cur, new = Mcur, Mnew
    for s in (1, 2, 4, 8, 16, 32, 64):
        nc.tensor.matmul(ps_m[:], sh_up[s][:], cur[:])
        V.tensor_tensor(ash[:], ps_m[:], idpad[s][:], OP.add)
        a2 = ash.rearrange("p (r k) -> p r k", r=2)       # a rows
        bp = cur[:, 1:5:3].rearrange("p r -> p r").unsqueeze(2).to_broadcast(
            [P, 2, 3])                                     # (bp1, bp2)
        bq = cur[:, 2:6:3].unsqueeze(2).to_broadcast([P, 2, 3])
        bd = cur[:, 0:4:3].unsqueeze(2)                    # [P, 2, 1]
        a1g = a2[:, 0:1, :].to_broadcast([P, 2, 3])        # (ad1, ap1, aq1)
        a2g = a2[:, 1:2, :].to_broadcast([P, 2, 3])        # (ad2, ap2, aq2)
        u1 = u1t.rearrange("p (r k) -> p r k", r=2)
        u2 = u2t.rearrange("p (r k) -> p r k", r=2)
        v = v6.rearrange("p (r k) -> p r k", r=2)
        nw = new.rearrange("p (r k) -> p r k", r=2)
        V.tensor_tensor(u1[:], a1g, bp, OP.mult)
        V.tensor_tensor(u2[:], a2g, bq, OP.mult)
        V.tensor_tensor(v[:], u1[:], u2[:], OP.add)
        V.tensor_tensor(nw[:, :, 0:1], v[:, :, 0:1], bd, OP.add)
        V.tensor_copy(nw[:, :, 1:3], v[:, :, 1:3])
        cur, new = new, cur
    # exclusive d: alpha0/beta0 per partition = d-cols of T^hat_{p-1}
    ps_d = psum.tile([P, 2], fp, tag="ps_small")
    nc.tensor.matmul(ps_d[:], sh_up[1][:], cur[:, 0:4:3])
    ab0 = pool.tile([P, 2], fp)   # (alpha0, beta0)
    V.tensor_copy(ab0[:], ps_d[:])

    # ---------- true per-chunk incoming states ----------
    # s_true(c) = s_w0(c) + alpha0*(s_w1(c)-s_w0(c)) + beta0*(s_w2(c)-s_w0(c))
    dl2 = pool.tile([P, 2 * C], fp)
    dl3 = pool.tile([P, 2 * C], fp)
    atr = pool.tile([P, 2 * C], fp)   # cols: [alpha_true (C), beta_true (C)]
    for comp, off in ((1, 0), (0, C)):  # alpha at k=1, beta at k=0
        s0 = S4[:, 0:1, 0:C, comp:comp + 1].rearrange("p a b c -> p (a b c)")
        s1 = S4[:, 1:2, 0:C, comp:comp + 1].rearrange("p a b c -> p (a b c)")
        s2 = S4[:, 2:3, 0:C, comp:comp + 1].rearrange("p a b c -> p (a b c)")
        V.tensor_tensor(dl2[:, off:off + C], s1, s0, OP.subtract)
        V.tensor_tensor(dl3[:, off:off + C], s2, s0, OP.subtract)
        V.scalar_tensor_tensor(atr[:, off:off + C], dl2[:, off:off + C],
                               ab0[:, 0:1], s0, OP.mult, OP.add)
        V.scalar_tensor_tensor(atr[:, off:off + C], dl3[:, off:off + C],
                               ab0[:, 1:2], atr[:, off:off + C], OP.mult, OP.add)

    # ---------- correction pass: y = y_zs + p*alpha_c + q*beta_c ----------
    yfin = pool.tile([P, W], fp)
    y3 = yfin.rearrange("p (c n) -> p c n", c=C)
    t1 = pool.tile([P, W], fp)
    t13 = t1.rearrange("p (c n) -> p c n", c=C)
    t2 = pool.tile([P, W], fp)
    t23 = t2.rearrange("p (c n) -> p c n", c=C)
    pv = ypq3[:, :, 7:6 + 3 * L:3]
    qv = ypq3[:, :, 8:6 + 3 * L:3]
    yzs = ypq3[:, :, 6:4 + 3 * L:3]
    alc = atr[:, 0:C].rearrange("p c -> p c").unsqueeze(2).to_broadcast([P, C, L])
    bec = atr[:, C:2 * C].unsqueeze(2).to_broadcast([P, C, L])
    V.tensor_tensor(t13[:], pv, alc, OP.mult)
    V.tensor_tensor(t23[:], qv, bec, OP.mult)
    V.tensor_tensor(y3[:], t13[:], yzs, OP.add)
    V.tensor_tensor(y3[:], y3[:], t23[:], OP.add)

    nc.sync.dma_start(d_y, yfin[:])


_NC_CACHE = None


def _get_nc():
    global _NC_CACHE
    if _NC_CACHE is None:
        _NC_CACHE = build_program()
    return _NC_CACHE


def kernel(noise_bursts, segment_ids, logits):
    from concourse.bass_utils import run_bass_kernel_spmd

    noise = np.ascontiguousarray(np.asarray(noise_bursts, dtype=np.float32))
    seg = np.ascontiguousarray(np.asarray(segment_ids).astype(np.int32))
    lg = np.ascontiguousarray(np.asarray(logits, dtype=np.float32))
    assert noise.shape == (B, T) and seg.shape == (B, T) and lg.shape == (B, T, 3)

    nc = _get_nc()
    in_maps = [
        {
            "noise": noise[r].reshape(P, W),
            "seg": seg[r].reshape(P, W),
            "logits": lg[r].reshape(P, 3 * W),
        }
        for r in range(B)
    ]
    res = run_bass_kernel_spmd(nc, in_maps, list(range(B)))
    out = np.stack([res.results[r]["y"].reshape(T) for r in range(B)])
    return out.astype(np.float32)
